# revision 23
# baseline (speedup 1.0000x reference)
"""Trainium2 Bass kernel for 2-layer GCN (nn_GCN_22866405884174).

Strategy (8 NeuronCores, dst-node sharding):
  out = A @ relu((A @ x) @ W1 + b1) @ W2 + b2   with A = D^-1/2 (Adj+I) D^-1/2
  (linear layers commute with aggregation, so each layer is: gather table
  rows by edge src + scatter-add by edge dst, then a small dense matmul).

  - Nodes sharded contiguously: core c owns dst nodes [c*12500, (c+1)*12500).
  - Table rows are QUARTER-INTERLEAVED: node (c, i) lives at table row
    q*SEC + c*QROWS + (i % QROWS) with q = i // QROWS.  Section q of the
    table therefore equals quarter q of every core's hidden shard, so the
    inter-layer AllGather splits into 4 sub-collectives (emitted right
    after the layer-1 block that completes their input quarter) and
    layer-2 gathers on section q start as soon as collective q lands.
  - Self-loops are NOT gathered: the self edge contributes the node's own
    table row, added via one identity matmul per block (layer 1) and a
    PE-transpose init of the layer-2 accumulator.
  - Layer tables are bf16 pre-scaled by dinv[src]; dinv[dst] applied
    post-aggregation on device.
  - Gather: SWDGE dma_gather, 1024-index single-packet chunks, 4 queues.
    Q7 descriptor generation (~2.8ns/row) and SDMA random-read drain are
    the critical resources; everything else is kept off the Pool engine
    and gather DMA bytes are minimized (one-hot scatter matrices S are
    GENERATED on the idle Vector engine via broadcast iota-compare
    instead of being streamed from HBM).
  - Scatter-add: PE matmul msg[slots,feat]^T @ S[slots,128dst] accumulated
    in PSUM; layer 1 per 512-dst block over all sections, layer 2 per
    (block, section) with SBUF f32 accumulation across sections.
  - Postprocessing (W matmul + relu/scale) fused per block: PSUM staging
    and activations on the Scalar(ACT) engine, table/output writes batched
    per block on the Sync queue.
"""

import numpy as np
import ml_dtypes

# ---------------- problem constants (hardcoded per contract) ----------------
N = 100000
E = 1600000
F_IN = 128
HID = 64
OUT_D = 10

NCORES = 8
NPC = N // NCORES           # 12500 nodes per core
SH = 12544                  # padded shard rows (98 * 128)
QROWS = SH // 4             # 3136 rows per quarter
SEC = NCORES * QROWS        # 25088 table rows per section (< int16 range)
NSEC = 4
NTOT = SEC * NSEC           # 100352
WDST = 64                   # dst window width
NWIN = (NPC + WDST - 1) // WDST   # 196 (last window = 20 dst)
WB = 8                      # windows per block
NBLK = (NWIN + WB - 1) // WB      # 25 (last block = 4 windows)
BCOLS = WB * WDST           # 512 dst cols per block
NG = SH // 128              # 98 node groups per shard
CHUNK = 1024                # gather chunk (single_packet: 16 engines x 64)

_CACHE = {}


def _srow(src):
    """Table row for global node id (quarter-interleaved layout)."""
    c = src // NPC
    i = src % NPC
    q = i // QROWS
    return q * SEC + c * QROWS + (i % QROWS)


# ============================ host preprocessing ============================

def _host_prep(edge_index):
    src = np.asarray(edge_index[0]).astype(np.int64)
    dst = np.asarray(edge_index[1]).astype(np.int64)
    # self-loops are handled on-device (own-row add), not as edges
    deg = (np.bincount(dst, minlength=N) + 1).astype(np.float32)
    dinv = 1.0 / np.sqrt(deg)

    srow = _srow(src)
    core = dst // NPC
    dloc = dst % NPC
    win = dloc // WDST
    sec = srow // SEC

    cellid = (core * NWIN + win) * NSEC + sec
    counts = np.bincount(cellid, minlength=NCORES * NWIN * NSEC).reshape(NCORES, NWIN, NSEC)
    # floor 128: guarantees a 128-slot group spans at most 2 windows, so
    # its dst cols fit one 128-wide S window.
    n_cell = np.maximum(counts.max(axis=0), 128)

    # ---- schedule: section-major slot streams, (block,sec) runs 128-aligned ----
    blocks = [[None] * NSEC for _ in range(NBLK)]
    sec_len = [0] * NSEC
    for s in range(NSEC):
        off = 0
        for b in range(NBLK):
            wlo, whi = b * WB, min(NWIN, (b + 1) * WB)
            cells = [int(n_cell[w, s]) for w in range(wlo, whi)]
            nbs = sum(cells)
            run = ((nbs + 127) // 128) * 128
            ngrp = run // 128
            bounds = np.cumsum([0] + cells)
            groups = []
            for j in range(ngrp):
                gslot = off + j * 128
                wi = int(np.searchsorted(bounds, j * 128, side="right") - 1)
                wi = min(wi, len(cells) - 1)
                base = min(wi * WDST, BCOLS - 128)
                groups.append((gslot // CHUNK, (gslot % CHUNK) // 128, base))
            blocks[b][s] = {
                "cells": cells, "nbs": nbs, "run": run, "ngrp": ngrp,
                "groups": groups, "soff": off,
            }
            off += run
        sec_len[s] = off

    # S group offsets in (s, b, j) order (section-major)
    TG = 0
    for s in range(NSEC):
        for b in range(NBLK):
            blocks[b][s]["gi"] = TG
            TG += blocks[b][s]["ngrp"]

    # idx tensor: section streams concatenated
    sec_coff = [0] * NSEC
    CIDX = 0
    for s in range(NSEC):
        sec_coff[s] = CIDX
        CIDX += sec_len[s] // 16

    chunks = [[] for _ in range(NSEC)]   # per section: chunk sizes
    for s in range(NSEC):
        rem = sec_len[s]
        while rem > 0:
            chunks[s].append(min(CHUNK, rem))
            rem -= min(CHUNK, rem)

    sort_key = (sec + NSEC * (win + NWIN * core))
    order = np.lexsort((dloc, sort_key))
    srow_s = srow[order]
    dloc_s = dloc[order]
    key_s = sort_key[order]

    idx_all = np.zeros((NCORES, 128, CIDX), dtype=np.int16)
    scol_all = np.full((NCORES, 128, TG), -1, dtype=np.int8)
    dinv_gt = np.zeros((NCORES, 128, NG), dtype=np.float32)

    cw_starts = np.searchsorted(key_s, np.arange(NCORES * NWIN * NSEC + 1))
    for c in range(NCORES):
        s_g = []
        s_p = []
        s_d = []
        for s in range(NSEC):
            # sentinel 0: a real row; pad slots have all-zero S columns so
            # whatever is gathered for them is multiplied by 0.
            stream = np.zeros(sec_len[s], dtype=np.int64)
            dcol_st = np.full(sec_len[s], -1, dtype=np.int64)
            for b in range(NBLK):
                info = blocks[b][s]
                off = info["soff"]
                wlo, whi = b * WB, min(NWIN, (b + 1) * WB)
                for wi, w in enumerate(range(wlo, whi)):
                    cid = (c * NWIN + w) * NSEC + s
                    a, e = cw_starts[cid], cw_starts[cid + 1]
                    cnt = e - a
                    stream[off:off + cnt] = srow_s[a:e] - s * SEC
                    dcol_st[off:off + cnt] = dloc_s[a:e] - b * BCOLS
                    off += info["cells"][wi]
                t0, t1 = info["soff"], info["soff"] + info["run"]
                t = np.arange(t0, t1)
                dc = dcol_st[t0:t1]
                real = dc >= 0
                j = (t - t0) // 128
                bases = np.array([g[2] for g in info["groups"]], dtype=np.int64)
                scol = dc - bases[j]
                if real.any():
                    assert scol[real].min() >= 0 and scol[real].max() < 128
                s_g.append(info["gi"] + j[real])
                s_p.append((t[real] - t0) % 128)
                s_d.append(scol[real])
            ncol = sec_len[s] // 16
            idx_all[c, :, sec_coff[s]:sec_coff[s] + ncol] = np.tile(
                stream.astype(np.int16).reshape(ncol, 16).T, (8, 1))
        scol_all[c, np.concatenate(s_p), np.concatenate(s_g)] = \
            np.concatenate(s_d).astype(np.int8)

        dpad = np.zeros(SH, dtype=np.float32)
        dpad[:NPC] = dinv[c * NPC:(c + 1) * NPC]
        dinv_gt[c] = dpad.reshape(NG, 128).T

    sched = {
        "blocks": blocks, "chunks": chunks, "sec_coff": sec_coff,
        "sec_len": sec_len, "CIDX": CIDX, "TG": TG,
    }
    return sched, idx_all, scol_all, dinv_gt, dinv


# ============================ device program ============================

def build_program(sched, b1_zero, b2_zero):
    import concourse.bass as bass
    import concourse.bacc as bacc
    import concourse.tile as tile
    import concourse.mybir as mybir

    CIDX = sched["CIDX"]
    TG = sched["TG"]
    blocks = sched["blocks"]
    chunks = sched["chunks"]
    sec_coff = sched["sec_coff"]
    GMAX_SB = max(blocks[b][s]["ngrp"] for b in range(NBLK) for s in range(NSEC))

    nc = bacc.Bacc(None, target_bir_lowering=False, debug=False, num_swdge_queues=4)
    f32 = mybir.dt.float32
    bf16 = mybir.dt.bfloat16
    fp8 = mybir.dt.float8e4
    i16 = mybir.dt.int16
    RELU = mybir.ActivationFunctionType.Relu
    COPY = mybir.ActivationFunctionType.Copy

    T1 = nc.dram_tensor("T1", [NTOT, F_IN], bf16, kind="ExternalInput")
    XST = nc.dram_tensor("XST", [128, SH], bf16, kind="ExternalInput")
    IDT = nc.dram_tensor("IDT", [128, 128], bf16, kind="ExternalInput")
    i8 = mybir.dt.int8
    IOTA = nc.dram_tensor("IOTA", [128, 128], i8, kind="ExternalInput")
    IDX = nc.dram_tensor("IDX", [128, CIDX], i16, kind="ExternalInput")
    SCOL = nc.dram_tensor("SCOL", [128, TG], i8, kind="ExternalInput")
    DINV = nc.dram_tensor("DINV", [128, NG], f32, kind="ExternalInput")
    DINV2 = nc.dram_tensor("DINV2", [128, NG], f32, kind="ExternalInput")
    W1T = nc.dram_tensor("W1T", [F_IN, HID], f32, kind="ExternalInput")
    B1T = nc.dram_tensor("B1T", [128, HID], f32, kind="ExternalInput")
    W2T = nc.dram_tensor("W2T", [HID, OUT_D], f32, kind="ExternalInput")
    B2T = nc.dram_tensor("B2T", [128, OUT_D], f32, kind="ExternalInput")
    OUTE = nc.dram_tensor("OUTE", [SH, OUT_D], f32, kind="ExternalOutput")

    t_local = nc.dram_tensor("t_local", [SH, F_IN], bf16)
    t_full = nc.dram_tensor("t_full", [NTOT, F_IN], bf16, addr_space="Shared")

    with tile.TileContext(nc) as tc:
        with (
            tc.tile_pool(name="resident", bufs=1) as rpool,
            tc.tile_pool(name="msg", bufs=34) as mpool,
            tc.tile_pool(name="sv", bufs=10) as spool,
            tc.tile_pool(name="xs", bufs=3) as xpool,
            tc.tile_pool(name="stage", bufs=2) as stpool,
            tc.tile_pool(name="post", bufs=3) as ppool,
            tc.tile_pool(name="psum", bufs=3, space="PSUM") as psum_pool,
            tc.tile_pool(name="psum2", bufs=3, space="PSUM") as psum_pool2,
            tc.tile_pool(name="psum3", bufs=2, space="PSUM") as psum_pool3,
        ):
            idx_t = rpool.tile([128, CIDX], i16)
            for part in range(8):
                c0 = (CIDX * part) // 8
                c1 = (CIDX * (part + 1)) // 8
                nc.sync.dma_start(idx_t[:, c0:c1], IDX[:, c0:c1])
            scol_t = rpool.tile([128, TG], i8)
            nc.sync.dma_start(scol_t[:], SCOL[:])
            iota_t = rpool.tile([128, 128], i8)
            nc.sync.dma_start(iota_t[:], IOTA[:])
            ident_t = rpool.tile([128, 128], bf16)
            nc.sync.dma_start(ident_t[:], IDT[:])
            dinv_t = rpool.tile([128, NG], f32)
            nc.sync.dma_start(dinv_t[:], DINV[:])
            dinv2_t = rpool.tile([128, NG], f32)
            nc.sync.dma_start(dinv2_t[:], DINV2[:])
            w1_t = rpool.tile([F_IN, HID], f32)
            nc.sync.dma_start(w1_t[:], W1T[:])
            b1_t = rpool.tile([128, HID], f32)
            nc.sync.dma_start(b1_t[:], B1T[:])
            w2_t = rpool.tile([HID, OUT_D], f32)
            nc.sync.dma_start(w2_t[:], W2T[:])
            b2_t = rpool.tile([128, OUT_D], f32)
            nc.sync.dma_start(b2_t[:], B2T[:])

            agg2 = rpool.tile([HID, SH], f32)
            t_selfT = rpool.tile([HID, SH], bf16)

            def gather_chunk(table, s, k, qn):
                csz = chunks[s][k]
                ng = (csz + 127) // 128
                msg = mpool.tile([128, CHUNK // 128, F_IN], bf16, tag="msg")
                nc.gpsimd.dma_gather(
                    msg[:, :ng, :],
                    table[s * SEC:(s + 1) * SEC, :],
                    idx_t[:, sec_coff[s] + k * (CHUNK // 16):
                          sec_coff[s] + k * (CHUNK // 16) + csz // 16],
                    csz,
                    csz,
                    F_IN,
                    single_packet=True,
                    queue_num=qn,
                )
                return msg

            def gen_sval(b, s):
                # one-hot S for (b, s) via broadcast iota-compare on DVE
                info = blocks[b][s]
                ngrp = info["ngrp"]
                gi = info["gi"]
                s_t = spool.tile([128, GMAX_SB, 128], fp8, tag="sval")
                ib = iota_t[:]
                in0 = bass.AP(ib.tensor, ib.offset,
                              [list(ib.ap[0]), [0, ngrp], list(ib.ap[1])])
                sb = scol_t[:, gi:gi + ngrp]
                in1 = bass.AP(sb.tensor, sb.offset,
                              [list(sb.ap[0]), list(sb.ap[1]), [0, 128]])
                nc.vector.tensor_tensor(
                    out=s_t[:, :ngrp, :], in0=in0, in1=in1,
                    op=mybir.AluOpType.is_equal,
                )
                return s_t

            def block_scatter(acc, msg_tiles, b, s, s_t, start, stop, mwid):
                info = blocks[b][s]
                for jn, (tk, jj, base) in enumerate(info["groups"]):
                    nc.tensor.matmul(
                        acc[:mwid, base:base + 128],
                        msg_tiles[(s, tk)][:, jj, :mwid],
                        s_t[:, jn, :],
                        start=(start and jn == 0),
                        stop=(stop and jn == len(info["groups"]) - 1),
                    )

            def postproc1(stage, b):
                # W1 matmul + relu(x*dinv)*dinv per 128-node group, batched
                # t_local write per block; also transposes the table rows
                # into agg2 as the layer-2 self-loop init.
                blo = b * BCOLS
                blen = min(BCOLS, NPC - blo)
                ngg = (blen + 127) // 128
                t_blk = ppool.tile([128, BCOLS // 128, HID], bf16, tag="tblk")
                for gg in range(ngg):
                    w = min(128, blen - gg * 128)
                    g = b * (BCOLS // 128) + gg
                    ph = psum_pool2.tile([128, HID], f32, tag="wout")
                    nc.tensor.matmul(
                        ph[:w, :], stage[:, gg * 128:gg * 128 + w], w1_t[:],
                        start=True, stop=True,
                    )
                    if b1_zero:
                        nc.scalar.activation(
                            t_blk[:w, gg, :], ph[:w, :], RELU,
                            scale=dinv2_t[:w, g:g + 1],
                        )
                    else:
                        tmp = ppool.tile([128, HID], f32, tag="tmp")
                        nc.vector.scalar_tensor_tensor(
                            out=tmp[:w, :], in0=ph[:w, :],
                            scalar=dinv_t[:w, g:g + 1], in1=b1_t[:w, :],
                            op0=mybir.AluOpType.mult, op1=mybir.AluOpType.add,
                        )
                        nc.scalar.activation(
                            t_blk[:w, gg, :], tmp[:w, :], RELU,
                            scale=dinv_t[:w, g:g + 1],
                        )
                    # layer-2 self-loop: t_selfT[:, nodes] = t_blk^T (added
                    # into the s=0 PSUM acc via an identity matmul later)
                    pst = psum_pool3.tile([HID, 128], f32, tag="pst")
                    nc.tensor.matmul(
                        pst[:, :w], t_blk[:w, gg, :], ident_t[:w, :w],
                        start=True, stop=True,
                    )
                    nc.scalar.activation(
                        t_selfT[:, blo + gg * 128:blo + gg * 128 + w],
                        pst[:, :w], COPY,
                    )
                if blen == BCOLS:
                    nc.sync.dma_start(
                        t_local[blo:blo + BCOLS, :HID].rearrange(
                            "(g p) c -> p g c", p=128),
                        t_blk[:],
                    )
                else:
                    for gg in range(ngg):
                        w = min(128, blen - gg * 128)
                        nc.sync.dma_start(
                            t_local[blo + gg * 128:blo + gg * 128 + w, :HID],
                            t_blk[:w, gg, :],
                        )

            def postproc2(b):
                blo = b * BCOLS
                blen = min(BCOLS, NPC - blo)
                ngg = (blen + 127) // 128
                o_blk = ppool.tile([128, BCOLS // 128, OUT_D], f32, tag="oblk")
                for gg in range(ngg):
                    w = min(128, blen - gg * 128)
                    g = b * (BCOLS // 128) + gg
                    po = psum_pool2.tile([128, HID], f32, tag="wout")
                    nc.tensor.matmul(
                        po[:w, :OUT_D],
                        agg2[:, blo + gg * 128:blo + gg * 128 + w],
                        w2_t[:],
                        start=True, stop=True,
                    )
                    if b2_zero:
                        nc.scalar.activation(
                            o_blk[:w, gg, :], po[:w, :OUT_D], COPY,
                            scale=dinv_t[:w, g:g + 1],
                        )
                    else:
                        nc.vector.scalar_tensor_tensor(
                            out=o_blk[:w, gg, :], in0=po[:w, :OUT_D],
                            scalar=dinv_t[:w, g:g + 1], in1=b2_t[:w, :OUT_D],
                            op0=mybir.AluOpType.mult, op1=mybir.AluOpType.add,
                        )
                if blen == BCOLS:
                    nc.sync.dma_start(
                        OUTE[blo:blo + BCOLS, :].rearrange(
                            "(g p) c -> p g c", p=128),
                        o_blk[:],
                    )
                else:
                    for gg in range(ngg):
                        w = min(128, blen - gg * 128)
                        nc.sync.dma_start(
                            OUTE[blo + gg * 128:blo + gg * 128 + w, :],
                            o_blk[:w, gg, :],
                        )

            # ---------------- layer 1 ----------------
            # Gather issue is interleaved with block processing in EMISSION
            # order so each quarter-AllGather can be emitted right after the
            # block whose t_local write completes its input quarter (deps are
            # annotated at emission time).  S tiles for the next block are
            # generated on DVE before this block's DVE copies so the engine
            # streams don't interlock.
            msg_tiles = {}
            qn = 0
            kmax = max(len(chunks[s]) for s in range(NSEC))
            kneed = [max((blocks[b][s]["soff"] + blocks[b][s]["run"] + CHUNK - 1)
                         // CHUNK for s in range(NSEC)) for b in range(NBLK)]
            issued = [0]

            def issue_upto(kk, table):
                nonlocal qn
                while issued[0] < kk:
                    k = issued[0]
                    for s in range(NSEC):
                        if k < len(chunks[s]):
                            msg_tiles[(s, k)] = gather_chunk(table, s, k, qn)
                            qn = (qn + 1) % 4
                    issued[0] += 1

            cmark = {6: 0, 12: 1, 18: 2, NBLK - 1: 3}
            sv_next = [gen_sval(0, s) for s in range(NSEC)]
            xs_next = xpool.tile([128, BCOLS], bf16, tag="xst")
            nc.scalar.dma_start(xs_next[:], XST[:, 0:BCOLS])
            for b in range(NBLK):
                issue_upto(min(kneed[min(b + 2, NBLK - 1)], kmax), T1)
                blen = min(BCOLS, NPC - b * BCOLS)
                sv_cur, xs_cur = sv_next, xs_next
                if b + 1 < NBLK:
                    sv_next = [gen_sval(b + 1, s) for s in range(NSEC)]
                    nblo = (b + 1) * BCOLS
                    nblen = min(BCOLS, NPC - nblo)
                    xs_next = xpool.tile([128, BCOLS], bf16, tag="xst")
                    nc.scalar.dma_start(
                        xs_next[:, :nblen], XST[:, nblo:nblo + nblen])
                acc = psum_pool.tile([128, BCOLS], f32, tag="acc")
                for s in range(NSEC):
                    block_scatter(acc, msg_tiles, b, s, sv_cur[s],
                                  start=(s == 0), stop=False, mwid=F_IN)
                # self-loop: acc[:, i] += own table column (identity matmul)
                nc.tensor.matmul(
                    acc[:, :blen], ident_t[:],
                    xs_cur[:, :blen], start=False, stop=True,
                )
                stage = stpool.tile([128, BCOLS], f32, tag="stage")
                nc.scalar.activation(stage[:, :blen], acc[:, :blen], COPY)
                postproc1(stage, b)
                if b in cmark:
                    q = cmark[b]
                    nc.gpsimd.collective_compute(
                        "AllGather",
                        mybir.AluOpType.bypass,
                        replica_groups=[list(range(NCORES))],
                        ins=[t_local[q * QROWS:(q + 1) * QROWS, :]],
                        outs=[t_full[q * SEC:(q + 1) * SEC, :]],
                    )

            # ---------------- layer 2 (section-major) ----------------
            for s in range(NSEC):
                kneed_s = [(blocks[b][s]["soff"] + blocks[b][s]["run"]
                            + CHUNK - 1) // CHUNK for b in range(NBLK)]
                nks = len(chunks[s])
                issued_s = 0
                sv_next = gen_sval(0, s)
                for b in range(NBLK):
                    kk = min(kneed_s[min(b + 3, NBLK - 1)], nks)
                    while issued_s < kk:
                        msg_tiles[(s, issued_s)] = gather_chunk(
                            t_full, s, issued_s, qn)
                        qn = (qn + 1) % 4
                        issued_s += 1
                    sv_cur = sv_next
                    if b + 1 < NBLK:
                        sv_next = gen_sval(b + 1, s)
                    blo = b * BCOLS
                    blen = min(BCOLS, NPC - blo)
                    acc = psum_pool.tile([128, BCOLS], f32, tag="acc")
                    block_scatter(acc, msg_tiles, b, s, sv_cur,
                                  start=True, stop=(s != 0), mwid=HID)
                    if s == 0:
                        # fold the self-loop term into the PSUM accumulation
                        nc.tensor.matmul(
                            acc[:HID, :blen], ident_t[:HID, :HID],
                            t_selfT[:, blo:blo + blen],
                            start=False, stop=True,
                        )
                        nc.vector.tensor_copy(
                            agg2[:, blo:blo + blen], acc[:HID, :blen])
                    else:
                        nc.vector.tensor_tensor(
                            out=agg2[:, blo:blo + blen],
                            in0=agg2[:, blo:blo + blen],
                            in1=acc[:HID, :blen], op=mybir.AluOpType.add,
                        )
                    if s == NSEC - 1:
                        postproc2(b)

    nc.compile()
    return nc


# ============================ entry point ============================

def prepare(x, edge_index, W1, b1, W2, b2):
    x = np.asarray(x, dtype=np.float32)
    W1 = np.asarray(W1, dtype=np.float32)
    b1 = np.asarray(b1, dtype=np.float32)
    W2 = np.asarray(W2, dtype=np.float32)
    b2 = np.asarray(b2, dtype=np.float32)

    sched, idx_all, scol_all, dinv_gt, dinv = _host_prep(edge_index)
    b1_zero = bool(np.all(b1 == 0))
    b2_zero = bool(np.all(b2 == 0))

    key = ("v7", sched["CIDX"], sched["TG"], b1_zero, b2_zero)
    if key in _CACHE:
        nc = _CACHE[key]
    else:
        nc = build_program(sched, b1_zero, b2_zero)
        _CACHE[key] = nc

    xs = x * dinv[:, None]
    T1 = np.zeros((NTOT, F_IN), dtype=ml_dtypes.bfloat16)
    nodes = np.arange(N, dtype=np.int64)
    T1[_srow(nodes)] = xs.astype(ml_dtypes.bfloat16)

    iota = np.tile(np.arange(128, dtype=np.int8), (128, 1))
    ident = np.eye(128, dtype=ml_dtypes.bfloat16)
    b1_tile = np.tile(b1[None, :], (128, 1)).astype(np.float32)
    b2_tile = np.tile(b2[None, :], (128, 1)).astype(np.float32)

    in_maps = []
    for c in range(NCORES):
        xst = np.zeros((128, SH), dtype=ml_dtypes.bfloat16)
        xst[:, :NPC] = xs[c * NPC:(c + 1) * NPC].T.astype(ml_dtypes.bfloat16)
        in_maps.append({
            "T1": T1,
            "XST": xst,
            "IDT": ident,
            "IOTA": iota,
            "IDX": np.ascontiguousarray(idx_all[c]),
            "SCOL": np.ascontiguousarray(scol_all[c]),
            "DINV": np.ascontiguousarray(dinv_gt[c]),
            "DINV2": np.ascontiguousarray(dinv_gt[c] ** 2),
            "W1T": W1,
            "B1T": b1_tile,
            "W2T": W2,
            "B2T": b2_tile,
        })
    return nc, in_maps


def kernel(x, edge_index, W1, b1, W2, b2):
    from concourse.bass_utils import run_bass_kernel_spmd

    nc, in_maps = prepare(x, edge_index, W1, b1, W2, b2)
    r = run_bass_kernel_spmd(nc, in_maps, core_ids=list(range(NCORES)))
    out = np.empty((N, OUT_D), dtype=np.float32)
    for c in range(NCORES):
        out[c * NPC:(c + 1) * NPC] = r.results[c]["OUTE"][:NPC]
    return out
